# revision 3
# baseline (speedup 1.0000x reference)
"""Trainium2 Bass kernel for MultiHeadLatentAttention (v2, bf16).

Reference computation (B=2, S=2048, HIDDEN=2048, 16 heads x 128, LATENT=512):
  q_lat = x @ Wq_d ; kv_lat = x @ Wkv_d
  q = split_heads(q_lat @ Wq_u) ; k = split_heads(kv_lat @ Wk_u) ; v = split_heads(kv_lat @ Wv_u)
  q, k = rope(q, k)
  out = softmax(causal(q k^T / sqrt(d))) @ v   -> merge heads -> @ Wo

Sharding: 8 cores = 2 batches (data parallel) x 4-way tensor parallel over
heads (4 heads/core).  Each core computes the full latents for its batch
(replicated within the 4-core group), the up-projections + attention for its
4 heads, and a partial output projection over its heads' slice of Wo's input
dim.  The host sums the 4 partials per batch (cheap elementwise add).

v2 changes vs v1:
  - all matmul operands in bf16 (PSUM accumulation stays fp32): enables fast
    weight loads on HW, removes the fp32r N>=256 constraint, halves DMA.
  - causal mask applied on the PE as a second accumulating matmul
    (stationary ltri, moving -BIG*I) instead of DVE tensor_adds.
  - diagonal score blocks narrowed to the true causal width (128-wide min).
  - per-head pipeline pools hoisted out of the head loop with bufs=2 so
    head h+1's up-projection + rope overlap head h's attention.
  - all weights loaded upfront on the ACT DMA queue (x slabs + y stores on
    the SP queue); Wo prefetched long before stage D.
"""

import sys
from contextlib import ExitStack

sys.path.insert(0, "/opt/trn_rl_repo")

import numpy as np
import ml_dtypes

import concourse.bass as bass
import concourse.mybir as mybir
import concourse.tile as tile
from concourse import bacc
from concourse.bass_utils import run_bass_kernel_spmd

HIDDEN = 2048
LATENT = 512
NUM_HEADS = 16
HEAD_DIM = 128
THETA = 10000.0
B = 2
S_FULL = 2048
N_CORES = 8
TP = 4  # tensor-parallel group size (heads 16 / 4 = 4 per core)
HPC = NUM_HEADS // TP  # heads per core
DSL = HPC * HEAD_DIM  # per-core head-dim slice width (512)

F32 = mybir.dt.float32
BF16 = mybir.dt.bfloat16
BF16_NP = ml_dtypes.bfloat16

NEG = -1.0e30
SCALE = 1.0 / np.sqrt(HEAD_DIM)


def build_nc(S=S_FULL, finalize=True, iters=1, stages="full", variant=""):
    """Build the single-core SPMD program (same program all 8 cores)."""
    nc = bacc.Bacc(None, target_bir_lowering=False)

    KC_H = HIDDEN // 128   # 16 contraction chunks for hidden dim
    KC_L = LATENT // 128   # 4 contraction chunks for latent dim
    NB = S // 512          # number of 512-wide seq blocks
    SC = S // 128          # number of 128-wide seq chunks

    xT = nc.dram_tensor("xT", [HIDDEN, S], BF16, kind="ExternalInput")
    wqd = nc.dram_tensor("wqd", [HIDDEN, LATENT], BF16, kind="ExternalInput")
    wkvd = nc.dram_tensor("wkvd", [HIDDEN, LATENT], BF16, kind="ExternalInput")
    wqu = nc.dram_tensor("wqu", [LATENT, DSL], BF16, kind="ExternalInput")
    wku = nc.dram_tensor("wku", [LATENT, DSL], BF16, kind="ExternalInput")
    wvu = nc.dram_tensor("wvu", [LATENT, DSL], BF16, kind="ExternalInput")
    wo = nc.dram_tensor("wo", [DSL, HIDDEN], BF16, kind="ExternalInput")
    cosd = nc.dram_tensor("cosd", [128, S], F32, kind="ExternalInput")
    sind = nc.dram_tensor("sind", [128, S], F32, kind="ExternalInput")
    permd = nc.dram_tensor("permd", [128, 128], BF16, kind="ExternalInput")
    onesd = nc.dram_tensor("onesd", [128, 1], BF16, kind="ExternalInput")
    ltrid = nc.dram_tensor("ltrid", [128, 128], BF16, kind="ExternalInput")
    negid = nc.dram_tensor("negid", [128, 128], BF16, kind="ExternalInput")
    y = nc.dram_tensor("y", [S, HIDDEN], F32, kind="ExternalOutput")

    unroll = 1
    if iters < 0:
        unroll, iters = -iters, 1
    with tile.TileContext(nc) as tc, ExitStack() as _es:
        if iters > 1:
            _es.enter_context(tc.For_i(0, iters, 1))
        for _u in range(unroll):
          # ---- persistent pools ----
          with ExitStack() as es:
            pool = lambda nm, bufs, **kw: es.enter_context(
                tc.tile_pool(name=nm, bufs=bufs, **kw))
            p_out = pool("p_out", 1)
            p_lat = pool("p_lat", 1)
            p_const = pool("p_const", 1)
            p_wu = pool("p_wu", 1)
            p_wo = pool("p_wo", 1)
            p_rope = pool("p_rope", 1)

            outT = p_out.tile([128, HPC, S], BF16)      # attention out, transposed
            latq = p_lat.tile([128, KC_L, S], BF16)     # q_latT
            latkv = p_lat.tile([128, KC_L, S], BF16)    # kv_latT
            ones_sb = p_const.tile([128, 1], BF16)
            perm_sb = p_const.tile([128, 128], BF16)
            ltri_sb = p_const.tile([128, 128], BF16)
            negi_sb = p_const.tile([128, 128], BF16)
            wqu_sb = p_wu.tile([128, KC_L, DSL], BF16)
            wku_sb = p_wu.tile([128, KC_L, DSL], BF16)
            wvu_sb = p_wu.tile([128, KC_L, DSL], BF16)
            wo_sb = p_wo.tile([128, HPC, HIDDEN], BF16)
            cos_sb = p_rope.tile([128, S], F32)
            sin_sb = p_rope.tile([128, S], F32)

            # ================= stage A: down projections =================
            with ExitStack() as es_a:
                pool_a = lambda nm, bufs, **kw: es_a.enter_context(
                    tc.tile_pool(name=nm, bufs=bufs, **kw))
                p_wd = pool_a("p_wd", 1)
                p_xt = pool_a("p_xt", 2)
                ps_a = pool_a("ps_a", 4, space="PSUM")
                wqd_sb = p_wd.tile([128, KC_H, LATENT], BF16)
                wkvd_sb = p_wd.tile([128, KC_H, LATENT], BF16)

                def load_w_col(w_sb, w_dram, m):
                    nc.scalar.dma_start(
                        out=w_sb[:, :, m * 128:(m + 1) * 128],
                        in_=w_dram.rearrange("(kc p) l -> p kc l", p=128)
                        [:, :, m * 128:(m + 1) * 128])

                # prefetch order on the ACT queue: wqd m=0 first so the PE
                # can start as soon as xslab 0 lands on the SP queue.
                load_w_col(wqd_sb, wqd, 0)
                load_w_col(wkvd_sb, wkvd, 0)
                for nh in range(S // 512):
                    xslab = p_xt.tile([128, KC_H, 512], BF16, tag="xslab")
                    nc.sync.dma_start(
                        out=xslab,
                        in_=xT.rearrange("(kc p) s -> p kc s", p=128)
                        [:, :, nh * 512:(nh + 1) * 512])
                    if nh == 0:
                        for m in range(1, KC_L):
                            load_w_col(wqd_sb, wqd, m)
                            load_w_col(wkvd_sb, wkvd, m)
                        # remaining weights + constants (ACT queue, after wd)
                        nc.scalar.dma_start(
                            out=wvu_sb,
                            in_=wvu.rearrange("(kc p) d -> p kc d", p=128))
                        nc.scalar.dma_start(
                            out=wqu_sb,
                            in_=wqu.rearrange("(kc p) d -> p kc d", p=128))
                        nc.scalar.dma_start(
                            out=wku_sb,
                            in_=wku.rearrange("(kc p) d -> p kc d", p=128))
                        nc.scalar.dma_start(out=cos_sb, in_=cosd[:, :])
                        nc.scalar.dma_start(out=sin_sb, in_=sind[:, :])
                        nc.scalar.dma_start(out=ones_sb, in_=onesd[:, :])
                        nc.scalar.dma_start(out=perm_sb, in_=permd[:, :])
                        nc.scalar.dma_start(out=ltri_sb, in_=ltrid[:, :])
                        nc.scalar.dma_start(out=negi_sb, in_=negid[:, :])
                        nc.scalar.dma_start(
                            out=wo_sb,
                            in_=wo.rearrange("(ic p) o -> p ic o", p=128))
                    ci = 0
                    for m in range(KC_L):
                        for w_sb, lat in ((wqd_sb, latq), (wkvd_sb, latkv)):
                            acc = ps_a.tile([128, 512], F32, tag="acc_a")
                            for kc in range(KC_H):
                                nc.tensor.matmul(
                                    acc,
                                    w_sb[:, kc, m * 128:(m + 1) * 128],
                                    xslab[:, kc, :],
                                    start=(kc == 0), stop=(kc == KC_H - 1))
                            dst = lat[:, m, nh * 512:(nh + 1) * 512]
                            if ci % 2 == 0:
                                nc.scalar.copy(dst, acc)
                            else:
                                nc.vector.tensor_copy(dst, acc)
                            ci += 1

            if stages == "a":
                nc.sync.dma_start(out=y[0:128, 0:S],
                                  in_=latq[:, 0, :].bitcast(mybir.dt.uint16))
                nc.sync.dma_start(out=y[128:256, 0:S],
                                  in_=latkv[:, 0, :].bitcast(mybir.dt.uint16))

            # ================= stage B0: v for all 4 heads ===============
            run_b = stages == "full"
            with ExitStack() as es_b:
                pool_b = lambda nm, bufs, **kw: es_b.enter_context(
                    tc.tile_pool(name=nm, bufs=bufs, **kw))
                p_v = pool_b("p_v", 1)
                with tc.tile_pool(name="ps_v", bufs=4, space="PSUM") as ps_v:
                    v_sb = p_v.tile([128, SC, DSL], BF16)
                    for sc in range(SC if run_b else 0):
                        acc = ps_v.tile([128, DSL], F32, tag="acc_v")
                        for kc in range(KC_L):
                            nc.tensor.matmul(
                                acc,
                                latkv[:, kc, sc * 128:(sc + 1) * 128],
                                wvu_sb[:, kc, :],
                                start=(kc == 0), stop=(kc == KC_L - 1))
                        if sc % 2 == 0:
                            nc.vector.tensor_copy(v_sb[:, sc, :], acc)
                        else:
                            nc.scalar.copy(v_sb[:, sc, :], acc)

                # ============ stages B/C per head: up-proj + attention ====
                if True:
                    p_head = pool_b("p_head", 2)
                    p_rt = pool_b("p_rt", 3)
                    p_at = pool_b("p_at", 6)
                    p_rb = pool_b("p_rb", 2)
                    ps_b = pool_b("ps_b", 1, space="PSUM")
                    ps_br = pool_b("ps_br", 1, space="PSUM")
                    ps_s = pool_b("ps_s", 3, space="PSUM")
                    ps_o = pool_b("ps_o", 2, space="PSUM")
                    ps_n = pool_b("ps_n", 1, space="PSUM")
                    for h in range(HPC if run_b else 0):
                        qT = p_head.tile([128, S], BF16, tag="qT")
                        kT = p_head.tile([128, S], BF16, tag="kT")
                        hs = h * 128

                        # ---- up-projection + rope for head h ----
                        for dst, w_sb, lat in (
                                (qT, wqu_sb, latq),
                                (kT, wku_sb, latkv)):
                            for nb in range(NB):
                                sl = slice(nb * 512, (nb + 1) * 512)
                                pa = ps_b.tile([128, 512], F32, tag="pa")
                                for kc in range(KC_L):
                                    nc.tensor.matmul(
                                        pa, w_sb[:, kc, hs:hs + 128],
                                        lat[:, kc, sl],
                                        start=(kc == 0),
                                        stop=(kc == KC_L - 1))
                                raw = p_rt.tile([128, 512], BF16, tag="raw")
                                nc.scalar.copy(raw, pa)
                                pr = ps_br.tile([128, 512], F32, tag="pr")
                                nc.tensor.matmul(pr, perm_sb, raw,
                                                 start=True, stop=True)
                                rt = p_rt.tile([128, 512], BF16, tag="rt")
                                nc.vector.tensor_mul(dst[:, sl], pa,
                                                     cos_sb[:, sl])
                                nc.vector.tensor_mul(rt, pr, sin_sb[:, sl])
                                nc.vector.tensor_add(dst[:, sl],
                                                     dst[:, sl], rt)

                        # ---- attention for head h ----
                        for qb in range(NB):
                            kb_hi = 4 * qb + 4
                            po = ps_o.tile([128, 512], F32, tag="po")
                            pn = ps_n.tile([1, 512], F32, tag="pn")
                            for kb in range(kb_hi):
                                j = kb - 4 * qb
                                off = max(j, 0) * 128
                                w = 512 - off
                                q0 = qb * 512 + off
                                ps = ps_s.tile([128, 512], F32, tag="ps")
                                nc.tensor.matmul(
                                    ps[:, 0:w],
                                    kT[:, kb * 128:(kb + 1) * 128],
                                    qT[:, q0:q0 + w],
                                    start=True, stop=(j < 0))
                                if j >= 0:
                                    # causal mask on first 128 cols:
                                    # += -BIG * [c < r]
                                    nc.tensor.matmul(
                                        ps[:, 0:128], ltri_sb, negi_sb,
                                        start=False, stop=True)
                                et = p_at.tile([128, 512], BF16, tag="et")
                                nc.scalar.activation(
                                    out=et[:, 0:w], in_=ps[:, 0:w],
                                    func=mybir.ActivationFunctionType.Exp,
                                    scale=float(SCALE))
                                nc.tensor.matmul(
                                    po[:, off:512],
                                    v_sb[:, kb, hs:hs + 128],
                                    et[:, 0:w],
                                    start=(kb == 0),
                                    stop=(kb == kb_hi - 1))
                                nc.tensor.matmul(
                                    pn[0:1, off:512],
                                    ones_sb[:, 0:1],
                                    et[:, 0:w],
                                    start=(kb == 0),
                                    stop=(kb == kb_hi - 1))
                            qsl = slice(qb * 512, (qb + 1) * 512)
                            rc = p_rb.tile([1, 512], F32, tag="rc")
                            nc.vector.reciprocal(rc, pn[0:1, :])
                            rb = p_rb.tile([128, 512], F32, tag="rb")
                            nc.gpsimd.partition_broadcast(rb, rc)
                            nc.vector.tensor_mul(outT[:, h, qsl], po, rb)

            # ================= stage D: output projection ================
            run_d = stages == "full"
            with ExitStack() as es_d:
                pool_d = lambda nm, bufs, **kw: es_d.enter_context(
                    tc.tile_pool(name=nm, bufs=bufs, **kw))
                p_fin = pool_d("p_fin", 3)
                ps_d = pool_d("ps_d", 4, space="PSUM")
                for sc in range(SC if run_d else 0):
                    fin = p_fin.tile([128, HIDDEN], F32, tag="fin")
                    for ob in range(HIDDEN // 512):
                        acc = ps_d.tile([128, 512], F32, tag="acc_d")
                        for ic in range(HPC):
                            nc.tensor.matmul(
                                acc,
                                outT[:, ic, sc * 128:(sc + 1) * 128],
                                wo_sb[:, ic, ob * 512:(ob + 1) * 512],
                                start=(ic == 0), stop=(ic == HPC - 1))
                        osl = slice(ob * 512, (ob + 1) * 512)
                        if ob % 2 == 0:
                            nc.scalar.copy(fin[:, osl], acc)
                        else:
                            nc.vector.tensor_copy(fin[:, osl], acc)
                    nc.sync.dma_start(
                        out=y[sc * 128:(sc + 1) * 128, :], in_=fin)

    if finalize:
        nc.finalize()
    return nc


# ---------------------------------------------------------------------------
# host-side helpers


def host_inputs(x, Wq_d, Wkv_d, Wq_u, Wk_u, Wv_u, Wo, S=S_FULL):
    """Build the 8 per-core input maps from full inputs."""
    x = np.asarray(x, dtype=np.float32)
    Wq_d = np.asarray(Wq_d, dtype=np.float32)
    Wkv_d = np.asarray(Wkv_d, dtype=np.float32)
    Wq_u = np.asarray(Wq_u, dtype=np.float32)
    Wk_u = np.asarray(Wk_u, dtype=np.float32)
    Wv_u = np.asarray(Wv_u, dtype=np.float32)
    Wo = np.asarray(Wo, dtype=np.float32)

    inv_freq = 1.0 / (THETA ** (np.arange(0, HEAD_DIM, 2, dtype=np.float64)
                                / HEAD_DIM))  # (64,)
    pos = np.arange(S, dtype=np.float64)
    ang = pos[None, :] * np.concatenate([inv_freq, inv_freq])[:, None]  # (128, S)
    COS = np.cos(ang).astype(np.float32)
    SIN = np.sin(ang).astype(np.float32)

    # signed permutation for rotate_half in [d, seq] layout:
    # out[m] = -in[m+64] for m<64 ; +in[m-64] for m>=64
    PERM = np.zeros((128, 128), dtype=np.float32)
    for m in range(64):
        PERM[m + 64, m] = -1.0
        PERM[m, m + 64] = 1.0

    # mask constants: scores_ps[r, c] += sum_p ltri[p, r] * negi[p, c]
    #               = NEG * [c < r]
    p_ = np.arange(128)
    LTRI = (p_[:, None] < p_[None, :]).astype(np.float32)   # [p, r] = p < r
    NEGI = (NEG * np.eye(128)).astype(np.float32)

    bf = lambda a: np.ascontiguousarray(a, dtype=np.float32).astype(BF16_NP)

    in_maps = []
    for core in range(N_CORES):
        b, tp = core // TP, core % TP
        sl = slice(tp * DSL, (tp + 1) * DSL)
        in_maps.append({
            "xT": bf(x[b, :S].T),
            "wqd": bf(Wq_d),
            "wkvd": bf(Wkv_d),
            "wqu": bf(Wq_u[:, sl]),
            "wku": bf(Wk_u[:, sl]),
            "wvu": bf(Wv_u[:, sl]),
            "wo": bf(Wo[sl, :]),
            "cosd": COS,
            "sind": SIN,
            "permd": bf(PERM),
            "onesd": bf(np.ones((128, 1), dtype=np.float32)),
            "ltrid": bf(LTRI),
            "negid": bf(NEGI),
        })
    return in_maps


def assemble(results, S=S_FULL):
    out = np.zeros((B, S, HIDDEN), dtype=np.float32)
    for core in range(N_CORES):
        out[core // TP] += results[core]["y"]
    return out


_NC_CACHE = {}


def kernel(x, Wq_d, Wkv_d, Wq_u, Wk_u, Wv_u, Wo):
    S = x.shape[1]
    if S not in _NC_CACHE:
        _NC_CACHE[S] = build_nc(S)
    nc = _NC_CACHE[S]
    in_maps = host_inputs(x, Wq_d, Wkv_d, Wq_u, Wk_u, Wv_u, Wo, S=S)

    res = run_bass_kernel_spmd(nc, in_maps, list(range(N_CORES)))
    return assemble(res.results, S=S)
